# revision 1
# baseline (speedup 1.0000x reference)
"""Trainium2 Bass kernel for nn_ContinousNormalizingFlowRHS.

Computes, for z in R^{B x Z} and scalar time t:
  h0 = tanh(W1*t + B1); h1 = tanh(einsum('knm,km->kn', W2, h0) + B2)
  w_in  = (W3_win  @ h1[0] + b3_win ).reshape(F, Z)
  w_out = (W3_wout @ h1[1] + b3_wout).reshape(F, Z)
  b     =  W3_b    @ h1[2] + b3_b
  gate  = sigmoid(W3_gate @ h1[3] + b3_gate)
  h = tanh(z @ w_in.T + b); dz = (h*gate) @ w_out / F
  trace = ((1-h^2)*gate) @ (sum(w_in*w_out,1)) / F
  out = concat([dz, -trace[:,None]], -1)

Strategy (8 NeuronCores, single SPMD launch):
  The dominant cost is streaming W3_win/W3_wout (268 MB each) for the
  matvecs, so those are sharded row-wise across the 8 cores (F-sharding).
  Each core's matvec work is further split between the PE (transposed
  bf16 slices as stationary weights, h1 column as the moving operand)
  and the DVE (natural-layout slices, multiply by a partition-broadcast
  h1 then reduce along the free axis), so neither engine is the
  bottleneck and the HBM stream rate binds.  Each core then runs the
  batch matmuls for the FULL batch against its local f-slice, producing
  partial dz/trace sums.  Two pipelined ReduceScatter(add) ops complete
  the sum over F and hand each core its own batch shard of the output.
"""

import sys
import types
import numpy as np
import ml_dtypes

BF = ml_dtypes.bfloat16

# problem sizes (hardcoded per contract)
Z = 128
N = 256
F = 2048
B = 8192
N_CORES = 8

PE_COLS = 128       # per matrix: f-columns computed on the PE (rest on DVE)
CHUNK_R = 4096      # W3 rows per streamed PE chunk ([128, 4096] bf16 tiles)
DVE_CC = 16         # f-columns per DVE chunk (2048 rows)
BC = 512            # batch columns per stage-B chunk (one PSUM bank)


def _ensure_ntff_hook():
    """run_bass_kernel_spmd(trace=True) under axon needs antenv.axon_hooks."""
    if 'antenv.axon_hooks' in sys.modules:
        return
    try:
        from trn_agent_boot.trn_boot import _ntff_profile_via_ctypes
        hook = _ntff_profile_via_ctypes('/opt/axon/libaxon_pjrt.so')
    except Exception:
        hook = None
    try:
        import antenv
    except Exception:
        return
    mod = types.ModuleType('antenv.axon_hooks')
    mod.get_axon_ntff_profile_hook = lambda: hook
    mod.set_axon_ntff_profile_hook = lambda h: None
    sys.modules['antenv.axon_hooks'] = mod
    antenv.axon_hooks = mod


def build_module(n_cores=N_CORES, b=B, f=F, pe_cols=PE_COLS, chunk_r=CHUNK_R,
                 bc=BC, debug=False):
    """Build the Bass module (SPMD program, one per core)."""
    import concourse.tile as tile
    from concourse import bacc, mybir

    F32 = mybir.dt.float32
    BF16 = mybir.dt.bfloat16
    ADD = mybir.AluOpType.add

    fl = f // n_cores            # local f count
    nfb = fl // 128              # local f blocks of 128
    rows_pe = pe_cols * 128      # rows of W3 handled by the PE
    dve_cols = fl - pe_cols
    rows_dve = dve_cols * 128
    n_pe_chunks = rows_pe // chunk_r
    rpc = chunk_r // 128         # w columns produced per PE chunk
    dcc = DVE_CC                 # f-columns per DVE chunk
    n_dve_chunks = dve_cols // dcc
    bl = b // n_cores            # output batch shard
    hw = bl // 2                 # reduce-scatter half width
    assert rows_pe % chunk_r == 0 and dve_cols % dcc == 0
    assert hw % bc == 0

    nc = bacc.Bacc("TRN2", target_bir_lowering=False, debug=debug,
                   num_devices=n_cores)

    def inp(name, shape, dt):
        return nc.dram_tensor(name, shape, dt, kind="ExternalInput").ap()

    t_ap = inp("t", [1, 1], F32)
    w1_ap = inp("w1c", [128, 8], F32)
    b1_ap = inp("b1c", [128, 8], F32)
    b2_ap = inp("b2c", [128, 8], F32)
    w2t_ap = inp("w2tc", [128, 2048], BF16)
    w3winT_ap = inp("w3winT_sl", [N, rows_pe], BF16)
    w3woutT_ap = inp("w3woutT_sl", [N, rows_pe], BF16)
    w3winN_ap = inp("w3winN_sl", [rows_dve // (dcc * 128) * 128, dcc * N], BF16)
    w3woutN_ap = inp("w3woutN_sl", [rows_dve // (dcc * 128) * 128, dcc * N], BF16)
    b3win_ap = inp("b3win_c", [128, fl], F32)
    b3wout_ap = inp("b3wout_c", [128, fl], F32)
    w3bT_ap = inp("w3bT_sl", [N, fl], BF16)
    w3gateT_ap = inp("w3gateT_sl", [N, fl], BF16)
    b3b_ap = inp("b3b_c", [128, nfb], F32)
    b3gate_ap = inp("b3gate_c", [128, nfb], F32)
    zt_ap = inp("ztb", [128, b], BF16)
    eye_ap = inp("eyeb", [128, 128], BF16)
    out_ap = nc.dram_tensor("out", [Z + 1, bl], F32, kind="ExternalOutput").ap()

    with tile.TileContext(nc) as tc:
        with tc.tile_pool(name="persist", bufs=1) as pp, \
             tc.tile_pool(name="stream", bufs=4) as sp, \
             tc.tile_pool(name="work", bufs=3) as wp, \
             tc.tile_pool(name="ps_h", bufs=2, space="PSUM") as ps_h, \
             tc.tile_pool(name="ps_dz", bufs=2, space="PSUM") as ps_dz, \
             tc.tile_pool(name="ps_t2", bufs=2, space="PSUM") as ps_t2, \
             tc.tile_pool(name="ps_prep", bufs=2, space="PSUM") as ps_prep, \
             tc.tile_pool(name="dram", bufs=1, space="DRAM") as dp:

            # ---- parameter nets (tiny) ----------------------------------
            t_bc = pp.tile([128, 1], F32, tag="tbc")
            nc.gpsimd.dma_start(t_bc[:], t_ap.broadcast_to([128, 1]))
            w1_sb = pp.tile([128, 8], F32, tag="w1")
            b1_sb = pp.tile([128, 8], F32, tag="b1")
            b2_sb = pp.tile([128, 8], F32, tag="b2")
            w2t_sb = pp.tile([128, 2048], BF16, tag="w2t")
            nc.gpsimd.dma_start(w1_sb[:], w1_ap[:])
            nc.gpsimd.dma_start(b1_sb[:], b1_ap[:])
            nc.gpsimd.dma_start(b2_sb[:], b2_ap[:])
            nc.gpsimd.dma_start(w2t_sb[:], w2t_ap[:])

            h0pre = pp.tile([128, 8], F32, tag="h0pre")
            nc.vector.tensor_scalar_mul(h0pre[:], w1_sb[:], t_bc[:, 0:1])
            nc.vector.tensor_add(h0pre[:], h0pre[:], b1_sb[:])
            h0_sb = pp.tile([128, 8], BF16, tag="h0")
            nc.scalar.activation(h0_sb[:], h0pre[:],
                                 mybir.ActivationFunctionType.Tanh)

            ps_h1 = ps_prep.tile([128, 8], F32, tag="prep")
            for k4 in range(4):
                for nb in range(2):
                    c = k4 * 2 + nb
                    for mb in range(2):
                        lhs = w2t_sb[:, k4 * 512 + mb * 256 + nb * 128:
                                     k4 * 512 + mb * 256 + nb * 128 + 128]
                        nc.tensor.matmul(ps_h1[:, c:c + 1], lhs,
                                         h0_sb[:, k4 * 2 + mb:k4 * 2 + mb + 1],
                                         start=(mb == 0), stop=(mb == 1))
            h1pre = pp.tile([128, 8], F32, tag="h1pre")
            h1_sb = pp.tile([128, 8], BF16, tag="h1")
            nc.vector.tensor_add(h1pre[:], ps_h1[:], b2_sb[:])
            nc.scalar.activation(h1_sb[:], h1pre[:],
                                 mybir.ActivationFunctionType.Tanh)
            # h1 -> DRAM in (net, n) order, then broadcast-load nets 0/1
            # replicated across partitions AND repeated dcc times along the
            # free dim (so the DVE multiply runs chunk-granular).
            h1_dram = dp.tile([8, 128], BF16, tag="h1d")
            nc.gpsimd.dma_start(h1_dram.rearrange("c n -> n c"), h1_sb[:])
            h1b = []
            for k4 in range(2):
                hb = pp.tile([128, dcc * N], BF16, tag=f"h1b{k4}")
                src = h1_dram.rearrange("c n -> (c n)")[k4 * N:(k4 + 1) * N]
                src = src.unsqueeze(0).unsqueeze(0)
                nc.gpsimd.dma_start(hb[:], src.broadcast_to([128, dcc, N]))
                h1b.append(hb)

            # ---- phase 1: sharded matvecs, split across PE and DVE ------
            b3win_sb = pp.tile([128, fl], F32, tag="b3win")
            b3wout_sb = pp.tile([128, fl], F32, tag="b3wout")
            nc.scalar.dma_start(b3win_sb[:], b3win_ap[:])
            nc.scalar.dma_start(b3wout_sb[:], b3wout_ap[:])

            w_inT_bf = pp.tile([128, fl], BF16, tag="winT")
            w_outT_bf = pp.tile([128, fl], BF16, tag="woutT")

            # PE part: columns [0, pe_cols) of each matrix
            for w3T_ap, bias_sb, dst, net in ((w3winT_ap, b3win_sb, w_inT_bf, 0),
                                              (w3woutT_ap, b3wout_sb, w_outT_bf, 1)):
                for c in range(n_pe_chunks):
                    tiles = []
                    for nb in range(2):
                        w3t = sp.tile([128, chunk_r], BF16, tag="w3chunk")
                        nc.sync.dma_start(
                            w3t[:], w3T_ap[nb * 128:(nb + 1) * 128,
                                           c * chunk_r:(c + 1) * chunk_r])
                        tiles.append(w3t)
                    pw = ps_prep.tile([128, rpc], F32, tag="prep")
                    for a in range(rpc):
                        for nb in range(2):
                            nc.tensor.matmul(
                                pw[:, a:a + 1],
                                tiles[nb][:, a * 128:(a + 1) * 128],
                                h1_sb[:, net * 2 + nb:net * 2 + nb + 1],
                                start=(nb == 0), stop=(nb == 1))
                    nc.vector.tensor_add(dst[:, c * rpc:(c + 1) * rpc], pw[:],
                                         bias_sb[:, c * rpc:(c + 1) * rpc])

            # DVE part: columns [pe_cols, fl) of each matrix, one chunk-wide
            # multiply + one 3-D reduce per dcc columns.
            for w3N_ap, bias_sb, dst, net in ((w3winN_ap, b3win_sb, w_inT_bf, 0),
                                              (w3woutN_ap, b3wout_sb, w_outT_bf, 1)):
                acc = pp.tile([128, max(dve_cols, 1)], F32, tag=f"dacc{net}")
                for c in range(n_dve_chunks):
                    w3n = sp.tile([128, dcc * N], BF16, tag="w3nat")
                    nc.scalar.dma_start(w3n[:],
                                        w3N_ap[c * 128:(c + 1) * 128, :])
                    prod = wp.tile([128, dcc * N], BF16, tag="prod")
                    nc.vector.tensor_mul(prod[:], w3n[:], h1b[net][:])
                    nc.vector.tensor_reduce(
                        acc[:, c * dcc:(c + 1) * dcc],
                        prod.rearrange("p (a n) -> p a n", a=dcc),
                        mybir.AxisListType.X, ADD)
                if dve_cols:
                    nc.vector.tensor_add(dst[:, pe_cols:fl], acc[:, 0:dve_cols],
                                         bias_sb[:, pe_cols:fl])

            # heads: b and gate (psum [f, fb] columns)
            b3b_sb = pp.tile([128, nfb], F32, tag="b3b")
            b3gate_sb = pp.tile([128, nfb], F32, tag="b3gate")
            nc.gpsimd.dma_start(b3b_sb[:], b3b_ap[:])
            nc.gpsimd.dma_start(b3gate_sb[:], b3gate_ap[:])
            b_sb = pp.tile([128, nfb], F32, tag="bh")
            gate_sb = pp.tile([128, nfb], F32, tag="gate")
            gpre = pp.tile([128, nfb], F32, tag="gpre")
            for w3hT_ap, bias_sb, dst, net in ((w3bT_ap, b3b_sb, b_sb, 2),
                                               (w3gateT_ap, b3gate_sb, gpre, 3)):
                w3ht = sp.tile([128, 2 * fl], BF16, tag="w3head")
                nc.scalar.dma_start(
                    w3ht[:], w3hT_ap.rearrange("(nb p) fl -> p nb fl", p=128))
                phd = ps_prep.tile([128, nfb], F32, tag="prep")
                for a in range(nfb):
                    for nb in range(2):
                        nc.tensor.matmul(
                            phd[:, a:a + 1],
                            w3ht[:, nb * fl + a * 128:nb * fl + (a + 1) * 128],
                            h1_sb[:, net * 2 + nb:net * 2 + nb + 1],
                            start=(nb == 0), stop=(nb == 1))
                nc.vector.tensor_add(dst[:], phd[:], bias_sb[:])
            nc.scalar.activation(gate_sb[:], gpre[:],
                                 mybir.ActivationFunctionType.Sigmoid)

            # ---- stage-B constants --------------------------------------
            zt_sb = pp.tile([128, b], BF16, tag="zt")
            nc.scalar.dma_start(zt_sb[:], zt_ap[:])
            eye_sb = pp.tile([128, 128], BF16, tag="eye")
            nc.gpsimd.dma_start(eye_sb[:], eye_ap[:])

            # transpose w_in/w_out to [f, z]; fold gate into w_out
            w_outg = pp.tile([128, nfb * 128], BF16, tag="woutg")
            w_in_fz = pp.tile([128, nfb * 128], BF16, tag="winfz")
            sg = pp.tile([128, nfb], F32, tag="sg")
            for fb in range(nfb):
                ptr = ps_prep.tile([128, 128], BF16, tag="prep")
                nc.tensor.transpose(ptr[:], w_outT_bf[:, fb * 128:(fb + 1) * 128],
                                    eye_sb[:])
                nc.vector.tensor_scalar_mul(w_outg[:, fb * 128:(fb + 1) * 128],
                                            ptr[:], gate_sb[:, fb:fb + 1])
                pti = ps_prep.tile([128, 128], BF16, tag="prep")
                nc.tensor.transpose(pti[:], w_inT_bf[:, fb * 128:(fb + 1) * 128],
                                    eye_sb[:])
                nc.vector.tensor_copy(w_in_fz[:, fb * 128:(fb + 1) * 128], pti[:])
                # s' = sum_z w_in[f,z] * w_out[f,z] * gate[f]
                prod = wp.tile([128, 128], F32, tag="sprod")
                nc.vector.tensor_mul(prod[:], w_in_fz[:, fb * 128:(fb + 1) * 128],
                                     w_outg[:, fb * 128:(fb + 1) * 128])
                nc.vector.tensor_reduce(sg[:, fb:fb + 1], prod[:],
                                        mybir.AxisListType.X, ADD)
            sg_bf = pp.tile([128, nfb], BF16, tag="sgbf")
            nc.vector.tensor_copy(sg_bf[:], sg[:])
            # cneg = -sum_f s' / F
            csum = pp.tile([1, 1], F32, tag="csum")
            nc.gpsimd.tensor_reduce(csum[:], sg[:], mybir.AxisListType.XYZWC, ADD)
            cneg = pp.tile([1, 1], F32, tag="cneg")
            nc.scalar.mul(cneg[:], csum[:], -1.0 / f)

            # ---- stage B: batch matmuls over local f slice --------------
            # half h of every core's [Z+1, bl] output reduces in its own
            # ReduceScatter so the first one overlaps remaining compute.
            cc_in = [dp.tile([n_cores, Z, hw], BF16, tag=f"ccin{h}",
                             name=f"ccin{h}") for h in range(2)]
            cc_out = [dp.tile([Z, hw], BF16, tag=f"ccout{h}",
                              name=f"ccout{h}") for h in range(2)]
            cc_tr_in = dp.tile([n_cores, bl], F32, tag="cctri", name="cctri")
            cc_tr_out = dp.tile([1, bl], F32, tag="cctro", name="cctro")
            for half in range(2):
                for kk in range(n_cores):
                    for j in range(hw // bc):
                        g0 = kk * bl + half * hw + j * bc
                        pdz = ps_dz.tile([128, bc], F32, tag="pdz")
                        pt2 = ps_t2.tile([1, bc], F32, tag="pt2")
                        for fb in range(nfb):
                            ph = ps_h.tile([128, bc], F32, tag="ph")
                            nc.tensor.matmul(ph[:],
                                             w_inT_bf[:, fb * 128:(fb + 1) * 128],
                                             zt_sb[:, g0:g0 + bc],
                                             start=True, stop=True)
                            h_bf = wp.tile([128, bc], BF16, tag="hbf")
                            nc.scalar.activation(
                                h_bf[:], ph[:],
                                mybir.ActivationFunctionType.Tanh,
                                bias=b_sb[:, fb:fb + 1])
                            h2_bf = wp.tile([128, bc], BF16, tag="h2bf")
                            nc.vector.tensor_mul(h2_bf[:], h_bf[:], h_bf[:])
                            nc.tensor.matmul(pdz[:],
                                             w_outg[:, fb * 128:(fb + 1) * 128],
                                             h_bf[:],
                                             start=(fb == 0), stop=(fb == nfb - 1))
                            nc.tensor.matmul(pt2[:], sg_bf[:, fb:fb + 1], h2_bf[:],
                                             start=(fb == 0), stop=(fb == nfb - 1))
                        dz_sb = wp.tile([128, bc], BF16, tag="dzsb")
                        nc.scalar.mul(dz_sb[:], pdz[:], 1.0 / f)
                        tr_sb = wp.tile([1, bc], F32, tag="trsb")
                        nc.scalar.activation(
                            tr_sb[:], pt2[:],
                            mybir.ActivationFunctionType.Identity,
                            bias=cneg[0:1, 0:1], scale=1.0 / f)
                        off = j * bc
                        nc.sync.dma_start(cc_in[half][kk, :, off:off + bc],
                                          dz_sb[:])
                        nc.sync.dma_start(
                            cc_tr_in[kk, half * hw + off:half * hw + off + bc]
                            .unsqueeze(0), tr_sb[:])
                nc.gpsimd.collective_compute(
                    "ReduceScatter", ADD,
                    replica_groups=[list(range(n_cores))],
                    ins=[cc_in[half].opt()], outs=[cc_out[half].opt()])
                nc.gpsimd.dma_start(out_ap[0:Z, half * hw:(half + 1) * hw],
                                    cc_out[half][:])
            nc.gpsimd.collective_compute(
                "ReduceScatter", ADD,
                replica_groups=[list(range(n_cores))],
                ins=[cc_tr_in.opt()], outs=[cc_tr_out.opt()])
            nc.gpsimd.dma_start(out_ap[Z:Z + 1, :], cc_tr_out[:])

    nc.compile()
    return nc


def host_prep(t, z_and_logpz, W1, B1, W2, B2, W3_win, b3_win,
              W3_wout, b3_wout, W3_b, b3_b, W3_gate, b3_gate,
              n_cores=N_CORES, b=B, f=F, pe_cols=PE_COLS):
    """Shard + lay out the numpy inputs into per-core in_maps."""
    fl = f // n_cores
    nfb = fl // 128
    rows = fl * Z
    rows_pe = pe_cols * 128

    dcc = DVE_CC

    def pack_nat(x):  # [rows_dve, N] -> [nch*128, dcc*N], partition-contiguous
        nch = x.shape[0] // (dcc * 128)
        return np.ascontiguousarray(
            x.reshape(nch, dcc, 128, N).transpose(0, 2, 1, 3)
            .reshape(nch * 128, dcc * N))

    def col8(x):  # [4, 256] -> [128, 8] with col = k*2 + nb
        return np.ascontiguousarray(
            np.asarray(x, np.float32).reshape(4, 2, 128).transpose(2, 0, 1)
            .reshape(128, 8))

    t_in = np.asarray(t, np.float32).reshape(1, 1)
    w1c = col8(np.asarray(W1, np.float32)[:, :, 0])
    b1c = col8(B1)
    b2c = col8(B2)
    # lhsT tile for h1 net: [m128, (k4, mb, n)] = W2[k4, n, mb*128+m128]
    w2tc = np.ascontiguousarray(
        np.asarray(W2, np.float32).transpose(0, 2, 1)        # [k, m, n]
        .reshape(4, 2, 128, 256).transpose(2, 0, 1, 3).reshape(128, 2048)).astype(BF)
    w3win_bf = np.asarray(W3_win, np.float32).astype(BF)
    w3wout_bf = np.asarray(W3_wout, np.float32).astype(BF)
    w3b_bf = np.asarray(W3_b, np.float32).astype(BF)
    w3gate_bf = np.asarray(W3_gate, np.float32).astype(BF)
    b3win = np.asarray(b3_win, np.float32)
    b3wout = np.asarray(b3_wout, np.float32)
    b3b = np.asarray(b3_b, np.float32)
    b3gate = np.asarray(b3_gate, np.float32)
    z = np.asarray(z_and_logpz, np.float32)[:, :Z]
    ztb = np.ascontiguousarray(z.T).astype(BF)
    eye = np.eye(128, dtype=np.float32).astype(BF)

    in_maps = []
    for k in range(n_cores):
        r0 = k * rows
        f0 = k * fl
        in_maps.append({
            "t": t_in, "w1c": w1c, "b1c": b1c, "b2c": b2c, "w2tc": w2tc,
            "w3winT_sl": np.ascontiguousarray(w3win_bf[r0:r0 + rows_pe].T),
            "w3woutT_sl": np.ascontiguousarray(w3wout_bf[r0:r0 + rows_pe].T),
            "w3winN_sl": pack_nat(w3win_bf[r0 + rows_pe:r0 + rows]),
            "w3woutN_sl": pack_nat(w3wout_bf[r0 + rows_pe:r0 + rows]),
            "b3win_c": np.ascontiguousarray(
                b3win[r0:r0 + rows].reshape(fl, 128).T),
            "b3wout_c": np.ascontiguousarray(
                b3wout[r0:r0 + rows].reshape(fl, 128).T),
            "w3bT_sl": np.ascontiguousarray(w3b_bf[f0:f0 + fl].T),
            "w3gateT_sl": np.ascontiguousarray(w3gate_bf[f0:f0 + fl].T),
            "b3b_c": np.ascontiguousarray(b3b[f0:f0 + fl].reshape(nfb, 128).T),
            "b3gate_c": np.ascontiguousarray(
                b3gate[f0:f0 + fl].reshape(nfb, 128).T),
            "ztb": ztb, "eyeb": eye,
        })
    return in_maps


_NC_CACHE = {}


def kernel(**inputs) -> np.ndarray:
    _ensure_ntff_hook()
    from concourse import bass_utils

    key = "full"
    if key not in _NC_CACHE:
        _NC_CACHE[key] = build_module()
    nc = _NC_CACHE[key]

    in_maps = host_prep(**inputs)
    res = bass_utils.run_bass_kernel_spmd(nc, in_maps, list(range(N_CORES)))
    bl = B // N_CORES
    out = np.empty((B, Z + 1), np.float32)
    for k in range(N_CORES):
        out[k * bl:(k + 1) * bl, :] = res.results[k]["out"].T
    return out



# revision 7
# speedup vs baseline: 1.6692x; 1.6692x over previous
"""Trainium2 Bass kernel for nn_ContinousNormalizingFlowRHS.

Computes, for z in R^{B x Z} and scalar time t:
  h0 = tanh(W1*t + B1); h1 = tanh(einsum('knm,km->kn', W2, h0) + B2)
  w_in  = (W3_win  @ h1[0] + b3_win ).reshape(F, Z)
  w_out = (W3_wout @ h1[1] + b3_wout).reshape(F, Z)
  b     =  W3_b    @ h1[2] + b3_b
  gate  = sigmoid(W3_gate @ h1[3] + b3_gate)
  h = tanh(z @ w_in.T + b); dz = (h*gate) @ w_out / F
  trace = ((1-h^2)*gate) @ (sum(w_in*w_out,1)) / F
  out = concat([dz, -trace[:,None]], -1)

Strategy (8 NeuronCores, single SPMD launch):
  Phase 1 (f-sharded): each core streams its 1/8 slice of W3_win/W3_wout
  (16.75 MB bf16 each) in 2 MB chunks and runs the matvec on the PE only
  (FWL stationary loads keep up with the 360 GB/s HBM stream; no DVE path,
  no broadcast DMAs).  The slice is processed in two f-halves; when a half
  finishes, its w_inT block, gate/F-folded w_outT block, and per-f scalars
  (sg, b) are packed into a 66 KB DRAM buffer and AllGathered (~0.5 MB
  total) while the next half still streams.
  Phase 2 (batch-sharded): each core computes its own 1024-row batch shard
  against the full gathered [F, Z] weights, so the output is written
  directly from each core -- no ReduceScatter tail.  Stage-2 work on the
  first gathered half overlaps the second half's weight streaming.
"""

import sys
import types
import numpy as np
import ml_dtypes

BF = ml_dtypes.bfloat16

# problem sizes (hardcoded per contract)
Z = 128
N = 256
F = 2048
B = 8192
N_CORES = 8

FL = F // N_CORES          # f per core (256)
HF = FL // 2               # f per half (128)
RH = HF * Z                # W3 rows per half per matrix (16384)
CW = 8192                  # W3 rows per streamed chunk
BL = B // N_CORES          # batch shard per core (1024)
BC = 512                   # batch columns per stage-2 chunk (one PSUM bank)


def _ensure_ntff_hook():
    """run_bass_kernel_spmd(trace=True) under axon needs antenv.axon_hooks."""
    if 'antenv.axon_hooks' in sys.modules:
        return
    try:
        from trn_agent_boot.trn_boot import _ntff_profile_via_ctypes
        hook = _ntff_profile_via_ctypes('/opt/axon/libaxon_pjrt.so')
    except Exception:
        hook = None
    try:
        import antenv
    except Exception:
        return
    mod = types.ModuleType('antenv.axon_hooks')
    mod.get_axon_ntff_profile_hook = lambda: hook
    mod.set_axon_ntff_profile_hook = lambda h: None
    sys.modules['antenv.axon_hooks'] = mod
    antenv.axon_hooks = mod


def build_module(n_cores=N_CORES, debug=False):
    """Build the Bass module (SPMD program, one per core)."""
    import concourse.tile as tile
    from concourse import bacc, mybir

    F32 = mybir.dt.float32
    BF16 = mybir.dt.bfloat16
    ADD = mybir.AluOpType.add
    BYPASS = mybir.AluOpType.bypass
    TANH = mybir.ActivationFunctionType.Tanh
    SIGM = mybir.ActivationFunctionType.Sigmoid

    ncc = CW // 128          # psum cols per chunk (64)
    n_chunks = RH // CW      # chunks per half per matrix (2)

    nc = bacc.Bacc("TRN2", target_bir_lowering=False, debug=debug,
                   num_devices=n_cores)

    def inp(name, shape, dt):
        return nc.dram_tensor(name, shape, dt, kind="ExternalInput").ap()

    t_ap = inp("t", [1, 1], F32)
    w1_ap = inp("w1c", [128, 8], F32)
    b1_ap = inp("b1c", [128, 8], F32)
    b2_ap = inp("b2c", [128, 8], F32)
    w2t_ap = inp("w2tc", [128, 2048], BF16)
    w3winT_ap = inp("w3winT_sl", [N, 2 * RH], BF16)
    w3woutT_ap = inp("w3woutT_sl", [N, 2 * RH], BF16)
    b3win_ap = inp("b3win_c", [128, FL], F32)
    b3wout_ap = inp("b3wout_c", [128, FL], F32)
    w3bT_ap = inp("w3bT_sl", [N, FL], BF16)
    w3gateT_ap = inp("w3gateT_sl", [N, FL], BF16)
    b3b_ap = inp("b3b_c", [128, 2], F32)
    b3gate_ap = inp("b3gate_c", [128, 2], F32)
    zt_ap = inp("ztb", [128, BL], BF16)
    eye_ap = inp("eyeb", [128, 128], BF16)
    ones_ap = inp("onesb", [128, 1], BF16)
    out_ap = nc.dram_tensor("out", [Z + 1, BL], F32, kind="ExternalOutput").ap()

    with tile.TileContext(nc) as tc:
        with tc.tile_pool(name="persist", bufs=1) as pp, \
             tc.tile_pool(name="stream", bufs=4) as sp, \
             tc.tile_pool(name="work", bufs=3) as wp, \
             tc.tile_pool(name="ps_h", bufs=2, space="PSUM") as ps_h, \
             tc.tile_pool(name="ps_dz", bufs=1, space="PSUM") as ps_dz, \
             tc.tile_pool(name="ps_t2", bufs=1, space="PSUM") as ps_t2, \
             tc.tile_pool(name="ps_prep", bufs=2, space="PSUM") as ps_prep, \
             tc.tile_pool(name="dram", bufs=1, space="DRAM") as dp:

            gbuf = [dp.tile([2 + 2 * 128, 128], BF16, tag=f"gbuf{x}",
                            name=f"gbuf{x}") for x in range(2)]
            gath = [dp.tile([n_cores * (2 + 2 * 128), 128], BF16,
                            tag=f"gath{x}", name=f"gath{x}") for x in range(2)]

            # ---- parameter nets (tiny) ----------------------------------
            t_bc = pp.tile([128, 1], F32, tag="tbc")
            nc.gpsimd.dma_start(t_bc[:], t_ap.broadcast_to([128, 1]))
            w1_sb = pp.tile([128, 8], F32, tag="w1")
            b1_sb = pp.tile([128, 8], F32, tag="b1")
            b2_sb = pp.tile([128, 8], F32, tag="b2")
            w2t_sb = pp.tile([128, 2048], BF16, tag="w2t")
            nc.gpsimd.dma_start(w1_sb[:], w1_ap[:])
            nc.gpsimd.dma_start(b1_sb[:], b1_ap[:])
            nc.gpsimd.dma_start(b2_sb[:], b2_ap[:])
            nc.gpsimd.dma_start(w2t_sb[:], w2t_ap[:])

            h0pre = pp.tile([128, 8], F32, tag="h0pre")
            nc.vector.tensor_scalar_mul(h0pre[:], w1_sb[:], t_bc[:, 0:1])
            nc.vector.tensor_add(h0pre[:], h0pre[:], b1_sb[:])
            h0_sb = pp.tile([128, 8], BF16, tag="h0")
            nc.scalar.activation(h0_sb[:], h0pre[:], TANH)

            ps_h1 = ps_prep.tile([128, 8], F32, tag="prep")
            for k4 in range(4):
                for nb in range(2):
                    c = k4 * 2 + nb
                    for mb in range(2):
                        lhs = w2t_sb[:, k4 * 512 + mb * 256 + nb * 128:
                                     k4 * 512 + mb * 256 + nb * 128 + 128]
                        nc.tensor.matmul(ps_h1[:, c:c + 1], lhs,
                                         h0_sb[:, k4 * 2 + mb:k4 * 2 + mb + 1],
                                         start=(mb == 0), stop=(mb == 1))
            h1pre = pp.tile([128, 8], F32, tag="h1pre")
            h1_sb = pp.tile([128, 8], BF16, tag="h1")
            nc.vector.tensor_add(h1pre[:], ps_h1[:], b2_sb[:])
            nc.scalar.activation(h1_sb[:], h1pre[:], TANH)

            # ---- small persistent loads ---------------------------------
            b3win_sb = pp.tile([128, FL], F32, tag="b3win")
            b3wout_sb = pp.tile([128, FL], F32, tag="b3wout")
            nc.gpsimd.dma_start(b3win_sb[:], b3win_ap[:])
            nc.gpsimd.dma_start(b3wout_sb[:], b3wout_ap[:])
            b3b_sb = pp.tile([128, 2], F32, tag="b3b")
            b3gate_sb = pp.tile([128, 2], F32, tag="b3gate")
            nc.gpsimd.dma_start(b3b_sb[:], b3b_ap[:])
            nc.gpsimd.dma_start(b3gate_sb[:], b3gate_ap[:])
            zt_sb = pp.tile([128, BL], BF16, tag="zt")
            nc.gpsimd.dma_start(zt_sb[:], zt_ap[:])
            eye_sb = pp.tile([128, 128], BF16, tag="eye")
            nc.gpsimd.dma_start(eye_sb[:], eye_ap[:])
            ones_sb = pp.tile([128, 1], BF16, tag="ones")
            nc.gpsimd.dma_start(ones_sb[:], ones_ap[:])

            # ---- heads: b and gate (partitions = f within block) --------
            b_sb = pp.tile([128, 2], F32, tag="bh")
            gate_sb = pp.tile([128, 2], F32, tag="gate")
            gpre = pp.tile([128, 2], F32, tag="gpre")
            for w3hT_ap, bias_sb, dst, net in ((w3bT_ap, b3b_sb, b_sb, 2),
                                               (w3gateT_ap, b3gate_sb, gpre, 3)):
                w3ht = wp.tile([128, 2 * FL], BF16, tag="w3head")
                nc.gpsimd.dma_start(
                    w3ht[:], w3hT_ap.rearrange("(nb p) fl -> p nb fl", p=128))
                phd = ps_prep.tile([128, 2], F32, tag="prep")
                for a in range(2):
                    for nb in range(2):
                        nc.tensor.matmul(
                            phd[:, a:a + 1],
                            w3ht[:, nb * FL + a * 128:nb * FL + (a + 1) * 128],
                            h1_sb[:, net * 2 + nb:net * 2 + nb + 1],
                            start=(nb == 0), stop=(nb == 1))
                nc.vector.tensor_add(dst[:], phd[:], bias_sb[:])
            nc.scalar.activation(gate_sb[:], gpre[:], SIGM)
            gateF = pp.tile([128, 2], F32, tag="gateF")
            nc.scalar.mul(gateF[:], gate_sb[:], 1.0 / F)

            # ---- phase 1: PE-only sharded matvecs -----------------------
            w_inT_loc = pp.tile([128, FL], BF16, tag="winTl")
            w_outT_loc = pp.tile([128, FL], BF16, tag="woutTl")

            def mv_chunk(x, c, w3T_ap, bias_sb, dst, net):
                off = x * RH + c * CW
                n0 = sp.tile([128, CW], BF16, tag="s0")
                nc.sync.dma_start(n0[:], w3T_ap[0:128, off:off + CW])
                n1 = sp.tile([128, CW], BF16, tag="s1")
                nc.scalar.dma_start(n1[:], w3T_ap[128:256, off:off + CW])
                pw = ps_prep.tile([128, ncc], F32, tag="prep")
                for a in range(ncc):
                    nc.tensor.matmul(pw[:, a:a + 1], n0[:, a * 128:(a + 1) * 128],
                                     h1_sb[:, net * 2:net * 2 + 1],
                                     start=True, stop=False)
                    nc.tensor.matmul(pw[:, a:a + 1], n1[:, a * 128:(a + 1) * 128],
                                     h1_sb[:, net * 2 + 1:net * 2 + 2],
                                     start=False, stop=True)
                col0 = x * HF + c * ncc
                nc.vector.tensor_add(dst[:, col0:col0 + ncc], pw[:],
                                     bias_sb[:, col0:col0 + ncc])

            MATS = ((w3winT_ap, b3win_sb, w_inT_loc, 0),
                    (w3woutT_ap, b3wout_sb, w_outT_loc, 1))

            w_inT_g = [None, None]
            w_outgT_g = [None, None]
            sgb_g = [None, None]
            b32_g = [None, None]

            def pack_half(x):
                c0 = x * HF
                ptr = ps_prep.tile([128, 128], BF16, tag="prep")
                nc.tensor.transpose(ptr[:], w_outT_loc[:, c0:c0 + 128], eye_sb[:])
                wog = wp.tile([128, 128], BF16, tag="wog")
                nc.vector.tensor_scalar_mul(wog[:], ptr[:], gateF[:, x:x + 1])
                pti = ps_prep.tile([128, 128], BF16, tag="prep")
                nc.tensor.transpose(pti[:], w_inT_loc[:, c0:c0 + 128], eye_sb[:])
                wif = wp.tile([128, 128], BF16, tag="wif")
                nc.vector.tensor_copy(wif[:], pti[:])
                prod = wp.tile([128, 128], F32, tag="sprod")
                nc.vector.tensor_mul(prod[:], wif[:], wog[:])
                sgf = wp.tile([128, 1], F32, tag="sgf")
                nc.vector.tensor_reduce(sgf[:], prod[:], mybir.AxisListType.X, ADD)
                sgb_pack = wp.tile([128, 2], BF16, tag="sgbp")
                nc.vector.tensor_copy(sgb_pack[:, 0:1], sgf[:])
                nc.vector.tensor_copy(sgb_pack[:, 1:2], b_sb[:, x:x + 1])
                nc.gpsimd.dma_start(gbuf[x][0:128, :], w_inT_loc[:, c0:c0 + 128])
                nc.gpsimd.dma_start(gbuf[x][128:256, :], wog[:])
                nc.gpsimd.dma_start(gbuf[x][256:258, :].rearrange("r f -> f r"),
                                    sgb_pack[:])
                nc.gpsimd.collective_compute(
                    "AllGather", BYPASS,
                    replica_groups=[list(range(n_cores))],
                    ins=[gbuf[x].opt()], outs=[gath[x].opt()])
                rX = gath[x].rearrange("(k r) f -> r k f", k=n_cores)
                w_inT_g[x] = pp.tile([128, n_cores * 128], BF16, tag=f"wing{x}", name=f"wing{x}")
                nc.gpsimd.dma_start(w_inT_g[x][:], rX[0:128])
                w_outgT_g[x] = pp.tile([128, n_cores * 128], BF16, tag=f"wogg{x}", name=f"wogg{x}")
                nc.gpsimd.dma_start(w_outgT_g[x][:], rX[128:256])
                vX = gath[x].rearrange("(k r) f -> f r k", k=n_cores)
                sgb_g[x] = pp.tile([128, n_cores], BF16, tag=f"sgg{x}", name=f"sgg{x}")
                nc.gpsimd.dma_start(sgb_g[x][:], vX[:, 256, :])
                bb = wp.tile([128, n_cores], BF16, tag=f"bbg{x}")
                nc.gpsimd.dma_start(bb[:], vX[:, 257, :])
                b32_g[x] = pp.tile([128, n_cores], F32, tag=f"b32g{x}", name=f"b32g{x}")
                nc.vector.tensor_copy(b32_g[x][:], bb[:])

            pdz = [ps_dz.tile([128, BC], F32, tag=f"pdz{j}", name=f"pdz{j}")
                   for j in range(2)]
            pt2 = [ps_t2.tile([1, BC], F32, tag=f"pt{j}", name=f"pt{j}")
                   for j in range(2)]

            def stage2(x, j):
                for i in range(n_cores):
                    ph = ps_h.tile([128, BC], F32, tag="ph")
                    nc.tensor.matmul(ph[:], w_inT_g[x][:, i * 128:(i + 1) * 128],
                                     zt_sb[:, j * BC:(j + 1) * BC],
                                     start=True, stop=True)
                    h_bf = wp.tile([128, BC], BF16, tag="hbf")
                    nc.scalar.activation(h_bf[:], ph[:], TANH,
                                         bias=b32_g[x][:, i:i + 1])
                    h2 = wp.tile([128, BC], BF16, tag="h2")
                    nc.vector.tensor_mul(h2[:], h_bf[:], h_bf[:])
                    first = (x == 0 and i == 0)
                    last = (x == 1 and i == n_cores - 1)
                    nc.tensor.matmul(pdz[j][:],
                                     w_outgT_g[x][:, i * 128:(i + 1) * 128],
                                     h_bf[:], start=first, stop=last)
                    nc.tensor.matmul(pt2[j][:], sgb_g[x][:, i:i + 1], h2[:],
                                     start=first, stop=last)

            # half A: stream + matvec, pack, AllGather
            for c in range(n_chunks):
                for m in MATS:
                    mv_chunk(0, c, *m)
            pack_half(0)

            # half B streams while stage-2 on half A fills PE idle time
            mv_chunk(1, 0, *MATS[0])
            mv_chunk(1, 0, *MATS[1])
            stage2(0, 0)
            mv_chunk(1, 1, *MATS[0])
            stage2(0, 1)
            mv_chunk(1, 1, *MATS[1])
            pack_half(1)
            stage2(1, 0)
            stage2(1, 1)

            # trace constant: cneg = -sum_f sg  (sg already has gate/F folded)
            sgs = wp.tile([128, 2], F32, tag="sgs")
            for x in range(2):
                nc.vector.tensor_reduce(sgs[:, x:x + 1], sgb_g[x][:],
                                        mybir.AxisListType.X, ADD)
            sgsum = wp.tile([128, 1], F32, tag="sgsum")
            nc.vector.tensor_reduce(sgsum[:], sgs[:], mybir.AxisListType.X, ADD)
            sgsum_bf = wp.tile([128, 1], BF16, tag="sgsbf")
            nc.vector.tensor_copy(sgsum_bf[:], sgsum[:])
            cps = ps_prep.tile([1, 1], F32, tag="prep")
            nc.tensor.matmul(cps[:], sgsum_bf[:], ones_sb[:],
                             start=True, stop=True)
            cneg = pp.tile([1, 1], F32, tag="cneg")
            nc.scalar.mul(cneg[:], cps[:], -1.0)

            # ---- outputs (direct, batch-sharded: no collective) ---------
            for j in range(2):
                dz_sb = wp.tile([128, BC], F32, tag="dzsb")
                nc.vector.tensor_copy(dz_sb[:], pdz[j][:])
                nc.sync.dma_start(out_ap[0:Z, j * BC:(j + 1) * BC], dz_sb[:])
                tr_sb = wp.tile([1, BC], F32, tag="trsb")
                nc.vector.tensor_scalar_add(tr_sb[:], pt2[j][:], cneg[0:1, 0:1])
                nc.gpsimd.dma_start(out_ap[Z:Z + 1, j * BC:(j + 1) * BC],
                                    tr_sb[:])

    nc.compile()
    return nc


def host_prep(t, z_and_logpz, W1, B1, W2, B2, W3_win, b3_win,
              W3_wout, b3_wout, W3_b, b3_b, W3_gate, b3_gate,
              n_cores=N_CORES):
    """Shard + lay out the numpy inputs into per-core in_maps."""
    rows = FL * Z  # W3 rows per core (32768)

    def col8(x):  # [4, 256] -> [128, 8] with col = k*2 + nb
        return np.ascontiguousarray(
            np.asarray(x, np.float32).reshape(4, 2, 128).transpose(2, 0, 1)
            .reshape(128, 8))

    t_in = np.asarray(t, np.float32).reshape(1, 1)
    w1c = col8(np.asarray(W1, np.float32)[:, :, 0])
    b1c = col8(B1)
    b2c = col8(B2)
    # lhsT tile for h1 net: [m128, (k4, mb, n)] = W2[k4, n, mb*128+m128]
    w2tc = np.ascontiguousarray(
        np.asarray(W2, np.float32).transpose(0, 2, 1)        # [k, m, n]
        .reshape(4, 2, 128, 256).transpose(2, 0, 1, 3).reshape(128, 2048)).astype(BF)
    w3win_bf = np.asarray(W3_win, np.float32).astype(BF)
    w3wout_bf = np.asarray(W3_wout, np.float32).astype(BF)
    w3b_bf = np.asarray(W3_b, np.float32).astype(BF)
    w3gate_bf = np.asarray(W3_gate, np.float32).astype(BF)
    b3win = np.asarray(b3_win, np.float32)
    b3wout = np.asarray(b3_wout, np.float32)
    b3b = np.asarray(b3_b, np.float32)
    b3gate = np.asarray(b3_gate, np.float32)
    z = np.asarray(z_and_logpz, np.float32)[:, :Z]
    ztb = np.ascontiguousarray(z.T).astype(BF)
    eye = np.eye(128, dtype=np.float32).astype(BF)
    ones = np.ones((128, 1), dtype=np.float32).astype(BF)

    in_maps = []
    for k in range(n_cores):
        r0 = k * rows
        f0 = k * FL
        in_maps.append({
            "t": t_in, "w1c": w1c, "b1c": b1c, "b2c": b2c, "w2tc": w2tc,
            "w3winT_sl": np.ascontiguousarray(w3win_bf[r0:r0 + rows].T),
            "w3woutT_sl": np.ascontiguousarray(w3wout_bf[r0:r0 + rows].T),
            "b3win_c": np.ascontiguousarray(
                b3win[r0:r0 + rows].reshape(FL, 128).T),
            "b3wout_c": np.ascontiguousarray(
                b3wout[r0:r0 + rows].reshape(FL, 128).T),
            "w3bT_sl": np.ascontiguousarray(w3b_bf[f0:f0 + FL].T),
            "w3gateT_sl": np.ascontiguousarray(w3gate_bf[f0:f0 + FL].T),
            "b3b_c": np.ascontiguousarray(b3b[f0:f0 + FL].reshape(2, 128).T),
            "b3gate_c": np.ascontiguousarray(
                b3gate[f0:f0 + FL].reshape(2, 128).T),
            "ztb": np.ascontiguousarray(ztb[:, k * BL:(k + 1) * BL]),
            "eyeb": eye, "onesb": ones,
        })
    return in_maps


_NC_CACHE = {}


def kernel(**inputs) -> np.ndarray:
    _ensure_ntff_hook()
    from concourse import bass_utils

    key = "full"
    if key not in _NC_CACHE:
        _NC_CACHE[key] = build_module()
    nc = _NC_CACHE[key]

    in_maps = host_prep(**inputs)
    res = bass_utils.run_bass_kernel_spmd(nc, in_maps, list(range(N_CORES)))
    out = np.empty((B, Z + 1), np.float32)
    for k in range(N_CORES):
        out[k * BL:(k + 1) * BL, :] = res.results[k]["out"].T
    return out
